# revision 1
# baseline (speedup 1.0000x reference)
"""Dilated (LongNet-style) attention kernel for 8 TRN2 NeuronCores.

Strategy (head-sharded, single AllToAll):
  - Core c owns heads {c, 8+c} (slot A / slot B). Slot A heads have branch-1
    dilation offset 0, slot B offset 1 (core-uniform), so the SPMD program is
    identical across cores; branch-2 offsets (c//4, 2+c//4) are selected via
    0/1 indicator scalars supplied as data.
  - Each core computes Q^T/K^T/V^T for its 128-feature slice from the FULL
    (host-transposed) inputs, runs all 14 of its segment attentions fully
    on-chip (scores transposed: [key, query] layout so the softmax needs no
    cross-partition reduction; Z row-sums come from an appended ones column
    in V), merges the three branches by scatter-accumulation, normalizes,
    then AllToAlls the merged head-block activations so each core can run
    the output projection for its own 512-position block.
  - All matmuls run in fp32r (TF32) at full PE rate. Jobs are emitted
    interleaved with the projection position-blocks they depend on so PE/ACT
    work overlaps the input streaming.
"""

import sys

if "/opt/trn_rl_repo" not in sys.path:
    sys.path.insert(0, "/opt/trn_rl_repo")

import numpy as np

import concourse.bacc as bacc
import concourse.bass as bass
import concourse.mybir as mybir
import concourse.tile as tile
from concourse import bass_utils

F32 = mybir.dt.float32
F32R = mybir.dt.float32r
AF = mybir.ActivationFunctionType

N_CORES = 8
E, L, H, D = 1024, 4096, 16, 64
KC = 8          # contraction chunks of 128 for the projections
PB = 512        # position block
NPB = L // PB   # 8
G = 1024        # compressed segment length (same for every branch)
VBW = 65        # V_both per-chunk width (64 feats + ones col)
NCH = 1         # a2a/outproj pipeline chunks

JOBS = [(0, 0), (0, 1), (1, 0), (0, 2), (0, 3), (1, 1), (2, 0)]


def _build(stage=6, dbg=False):
    """stage: 1=proj only, 2=+b2 copies, 3=+job b0s0, 4=+all jobs,
    5=+normalization, 6=full (a2a+outproj). dbg adds intermediate outputs."""
    nc = bacc.Bacc("TRN2", target_bir_lowering=False, debug=False,
                   num_devices=N_CORES)

    qT = nc.dram_tensor("qT", [E, L], F32R, kind="ExternalInput")
    kT = nc.dram_tensor("kT", [E, L], F32R, kind="ExternalInput")
    vT = nc.dram_tensor("vT", [E, L], F32R, kind="ExternalInput")
    wq = nc.dram_tensor("wq", [E, 128], F32R, kind="ExternalInput")
    wk = nc.dram_tensor("wk", [E, 128], F32R, kind="ExternalInput")
    wv = nc.dram_tensor("wv", [E, 128], F32R, kind="ExternalInput")
    wo = nc.dram_tensor("wo", [2, 128, 4 * E], F32R, kind="ExternalInput")
    bq = nc.dram_tensor("bq", [128, 1], F32, kind="ExternalInput")
    bk = nc.dram_tensor("bk", [128, 1], F32, kind="ExternalInput")
    bo8 = nc.dram_tensor("bo8", [128, 8], F32, kind="ExternalInput")
    ind2 = nc.dram_tensor("ind2", [2, 128], F32R, kind="ExternalInput")
    eyer = nc.dram_tensor("eyer", [128, 128], F32R, kind="ExternalInput")
    ones16 = nc.dram_tensor("ones16", [128, 16], F32R, kind="ExternalInput")
    wsel = nc.dram_tensor("wsel", [128, 2], F32, kind="ExternalInput")

    outT = nc.dram_tensor("outT", [E, PB], F32, kind="ExternalOutput")
    if dbg:
        dbg_qt = nc.dram_tensor("dbg_qt", [128, L], F32, kind="ExternalOutput")
        dbg_kt = nc.dram_tensor("dbg_kt", [128, L], F32, kind="ExternalOutput")
        dbg_vt = nc.dram_tensor("dbg_vt", [128, L], F32, kind="ExternalOutput")
        dbg_q2 = nc.dram_tensor("dbg_q2", [128, G], F32, kind="ExternalOutput")
        dbg_acc = nc.dram_tensor("dbg_acc", [128, L], F32, kind="ExternalOutput")
        dbg_zz = nc.dram_tensor("dbg_zz", [65, L], F32, kind="ExternalOutput")

    a2a_warm_in = nc.dram_tensor("a2a_warm_in", [8, 1, 64], F32R)
    a2a_warm_out = nc.dram_tensor("a2a_warm_out", [8, 1, 64], F32R)
    a2a_in = [nc.dram_tensor(f"a2a_in{h}", [8, 128, PB // NCH], F32R)
              for h in range(NCH)]
    a2a_out = [nc.dram_tensor(f"a2a_out{h}", [8, 128, PB // NCH], F32R)
               for h in range(NCH)]

    import contextlib

    def _emit(tc, ctx):
        pin = ctx.enter_context(tc.tile_pool(name="pin", bufs=3))
        persist = ctx.enter_context(tc.tile_pool(name="persist", bufs=1))
        vpool = ctx.enter_context(tc.tile_pool(name="vpool", bufs=2))
        epool = ctx.enter_context(tc.tile_pool(name="epool", bufs=4))
        opool = ctx.enter_context(tc.tile_pool(name="opool", bufs=2))
        psw = ctx.enter_context(tc.tile_pool(name="psw", bufs=2, space="PSUM"))
        pso = ctx.enter_context(tc.tile_pool(name="pso", bufs=2, space="PSUM"))

        # ---- small constants ----
        wq_sb = persist.tile([128, KC * 128], F32R, tag="wq")
        wk_sb = persist.tile([128, KC * 128], F32R, tag="wk")
        wv_sb = persist.tile([128, KC * 128], F32R, tag="wv")
        for w_d, w_t in ((wq, wq_sb), (wk, wk_sb), (wv, wv_sb)):
            wr = w_d.rearrange("(kc p) f -> kc p f", p=128)
            for kc in range(KC):
                nc.sync.dma_start(w_t[:, kc * 128:(kc + 1) * 128], wr[kc])
        bq_sb = persist.tile([128, 1], F32, tag="bq")
        bk_sb = persist.tile([128, 1], F32, tag="bk")
        bo_sb = persist.tile([128, 8], F32, tag="bo")
        ind_sb = persist.tile([2, 128], F32R, tag="ind")
        eye_sb = persist.tile([128, 128], F32R, tag="eye")
        on_sb = persist.tile([128, 16], F32R, tag="on")
        ws_sb = persist.tile([128, 2], F32, tag="ws")
        nc.sync.dma_start(bq_sb[:], bq[:])
        nc.sync.dma_start(bk_sb[:], bk[:])
        nc.sync.dma_start(bo_sb[:], bo8[:])
        nc.sync.dma_start(ind_sb[:], ind2[:])
        nc.sync.dma_start(eye_sb[:], eyer[:])
        nc.sync.dma_start(on_sb[:], ones16[:])
        nc.sync.dma_start(ws_sb[:], wsel[:])

        QT = persist.tile([128, L], F32R, tag="QT")
        KT = persist.tile([128, L], F32R, tag="KT")
        VT = persist.tile([128, L], F32R, tag="VT")
        QT2 = persist.tile([128, G], F32R, tag="QT2")
        KT2 = persist.tile([128, G], F32R, tag="KT2")
        VT2 = persist.tile([128, G], F32R, tag="VT2")
        acc = persist.tile([128, L], F32, tag="acc")
        zz = persist.tile([65, L], F32, tag="zz")

        streams = (
            ("k", kT, wk_sb, KT, bk_sb),
            ("v", vT, wv_sb, VT, None),
            ("q", qT, wq_sb, QT, bq_sb),
        )

        def proj_pb(pb):
            for name, x_d, w_t, dst, bias in streams:
                xin = pin.tile([128, KC * PB], F32R, tag="xin")
                xr = x_d.rearrange("(kc p) l -> kc p l", p=128)
                for kc in range(KC):
                    eng = (nc.sync, nc.gpsimd, nc.scalar)[kc % 3]
                    eng.dma_start(
                        xin[:, kc * PB:(kc + 1) * PB],
                        xr[kc][:, pb * PB:(pb + 1) * PB],
                    )
                pt = psw.tile([128, 1024], F32, tag="w")
                for kc in range(KC):
                    nc.tensor.matmul(
                        pt[:, 0:PB],
                        w_t[:, kc * 128:(kc + 1) * 128],
                        xin[:, kc * PB:(kc + 1) * PB],
                        start=(kc == 0), stop=(kc == KC - 1),
                    )
                dslice = dst[:, pb * PB:(pb + 1) * PB]
                if bias is not None:
                    nc.vector.tensor_scalar_add(dslice, pt[:, 0:PB], bias[:])
                else:
                    nc.vector.tensor_copy(dslice, pt[:, 0:PB])

        def b2_copies():
            # slot A picks dense offset 0 or 1, slot B picks 2 or 3, via
            # 0/1 indicators in ws_sb (core-uniform instruction stream).
            for src, dst in ((QT, QT2), (KT, KT2), (VT, VT2)):
                for slot in range(2):
                    p0, p1 = 64 * slot, 64 * slot + 64
                    o0 = 2 * slot
                    nc.vector.tensor_scalar_mul(
                        dst[p0:p1, :], src[p0:p1, o0::4], ws_sb[p0:p1, 0:1]
                    )
                    nc.vector.scalar_tensor_tensor(
                        dst[p0:p1, :], src[p0:p1, o0 + 1::4],
                        ws_sb[p0:p1, 1:2], dst[p0:p1, :],
                        mybir.AluOpType.mult, mybir.AluOpType.add,
                    )

        def kq_slice(br, seg, slot, t, lo, size):
            if br == 0:
                base = 1024 * seg + lo
                return t[slot * 64:(slot + 1) * 64, base:base + size]
            if br == 1:
                base = 2048 * seg + 2 * lo + slot
                return t[slot * 64:(slot + 1) * 64,
                         base:base + 2 * size - slot:2]
            return t[slot * 64:(slot + 1) * 64, lo:lo + size]

        def mm_ranges(jc):
            if jc <= 3:
                return [(128 * jc, 512 - 128 * jc), (512, 512)]
            return [(128 * jc, 1024 - 128 * jc)]

        def job(br, seg):
            kt_src = KT2 if br == 2 else KT
            qt_src = QT2 if br == 2 else QT
            # -- V_both prep --
            vb = vpool.tile([128, 2 * 8 * VBW], F32R, tag="vb")
            nc.vector.tensor_copy(vb[:, 64::VBW], on_sb[:])
            for jc in range(8):
                if br == 0:
                    tp = psw.tile([128, 1024], F32R, tag="w")
                    src = VT[:, 1024 * seg + 128 * jc:1024 * seg + 128 * (jc + 1)]
                    nc.tensor.transpose(tp[:, 0:128], src, eye_sb[:])
                    dst = vb[:].rearrange(
                        "p (s jj t) -> p s jj t", s=2, jj=8
                    )[:, :, jc, 0:64]
                    srcp = tp[:, 0:128].rearrange("p (s r) -> p s r", s=2)
                    nc.vector.tensor_copy(dst, srcp)
                else:
                    for slot in range(2):
                        tp = psw.tile([128, 1024], F32R, tag="w")
                        if br == 1:
                            base = 2048 * seg + 256 * jc + slot
                            src = VT[slot * 64:(slot + 1) * 64,
                                     base:base + 256 - slot:2]
                        else:
                            src = VT2[slot * 64:(slot + 1) * 64,
                                      128 * jc:128 * (jc + 1)]
                        nc.tensor.transpose(
                            tp[:, 0:64], src,
                            eye_sb[slot * 64:(slot + 1) * 64,
                                   slot * 64:(slot + 1) * 64],
                        )
                        nc.vector.tensor_copy(
                            vb[:, slot * 8 * VBW + jc * VBW:
                               slot * 8 * VBW + jc * VBW + 64],
                            tp[:, 0:64],
                        )

            o_ps_a = pso.tile([128, 1024], F32, tag="o")
            o_ps_b = pso.tile([128, 1024], F32, tag="o")
            o_ps = [o_ps_a, o_ps_b]

            for jc in range(8):
                s_ps_a = psw.tile([128, 1024], F32, tag="w")
                s_ps_b = psw.tile([128, 1024], F32, tag="w")
                s_ps = [s_ps_a, s_ps_b]
                for slot in range(2):
                    for lo, size in mm_ranges(jc):
                        lhs = kq_slice(br, seg, slot, kt_src, 128 * jc, 128)
                        rhs = kq_slice(br, seg, slot, qt_src, lo, size)
                        nc.tensor.matmul(
                            s_ps[slot][:, lo:lo + size], lhs, rhs,
                            start=True, stop=True,
                            tile_position=(slot * 64, 0),
                        )
                e_t = [None, None]
                for slot in range(2):
                    c0 = 128 * jc
                    e = epool.tile([128, 1024], F32R, tag="e")  # noqa
                    nc.scalar.activation(
                        e[:, c0:1024], s_ps[slot][:, c0:1024], AF.Exp
                    )
                    nc.gpsimd.affine_select(
                        e[:, c0:c0 + 128], e[:, c0:c0 + 128],
                        pattern=[[1, 128]],
                        compare_op=mybir.AluOpType.is_ge,
                        fill=0.0, base=0, channel_multiplier=-1,
                    )
                    e_t[slot] = e
                for slot in range(2):
                    for lo, size in mm_ranges(jc):
                        nc.tensor.matmul(
                            o_ps[slot][0:VBW, lo:lo + size],
                            vb[:, slot * 8 * VBW + jc * VBW:
                               slot * 8 * VBW + (jc + 1) * VBW],
                            e_t[slot][:, lo:lo + size],
                            start=(jc == 0), stop=(jc == 7),
                        )

            # -- merge into acc / zz (slot B copies on ACT to offload DVE) --
            for slot in range(2):
                op = o_ps[slot]
                po = slot * 64
                zr = 64 * slot
                if br == 0:
                    sl_ = slice(1024 * seg, 1024 * (seg + 1))
                    if slot == 0:
                        nc.vector.tensor_copy(acc[po:po + 64, sl_], op[0:64, :])
                        nc.vector.tensor_copy(zz[zr:zr + 1, sl_], op[64:65, :])
                    else:
                        nc.scalar.copy(acc[po:po + 64, sl_], op[0:64, :])
                        nc.scalar.copy(zz[zr:zr + 1, sl_], op[64:65, :])
                elif br == 1:
                    ac = acc[po:po + 64, 2048 * seg + slot:2048 * (seg + 1):2]
                    nc.vector.tensor_add(ac, ac, op[0:64, :])
                    zc = zz[zr:zr + 1, 2048 * seg + slot:2048 * (seg + 1):2]
                    nc.vector.tensor_add(zc, zc, op[64:65, :])
                else:
                    o0 = 2 * slot
                    for dd in range(2):
                        ac = acc[po:po + 64, o0 + dd::4]
                        nc.vector.scalar_tensor_tensor(
                            ac, op[0:64, :], ws_sb[po:po + 64, dd:dd + 1],
                            ac, mybir.AluOpType.mult, mybir.AluOpType.add,
                        )
                        zc = zz[zr:zr + 1, o0 + dd::4]
                        nc.vector.scalar_tensor_tensor(
                            zc, op[64:65, :], ws_sb[zr:zr + 1, dd:dd + 1],
                            zc, mybir.AluOpType.mult, mybir.AluOpType.add,
                        )

        # ================= emission order =================
        proj_pb(0)
        proj_pb(1)
        # tiny dummy collective: absorbs the ~30us first-collective setup
        # cost while the projections stream. Emitted after the first two
        # position blocks so their gpsimd-queue DMA chunks are not stalled
        # behind the collective trigger+wait.
        if stage >= 6:
            for rr in range(8):
                nc.sync.dma_start(a2a_warm_in[rr][0:1, 0:16], on_sb[0:1, 0:16])
            nc.gpsimd.collective_compute(
                "AllToAll", mybir.AluOpType.bypass,
                replica_groups=[list(range(8))],
                ins=[a2a_warm_in[:]], outs=[a2a_warm_out[:]],
            )
        if dbg and stage <= 1:
            for pbx in range(2, NPB):
                proj_pb(pbx)
            nc.sync.dma_start(dbg_qt[:], QT[:].bitcast(F32))
            nc.sync.dma_start(dbg_kt[:], KT[:].bitcast(F32))
            nc.sync.dma_start(dbg_vt[:], VT[:].bitcast(F32))
            return
        if stage >= 3:
            job(0, 0)
        proj_pb(2)
        proj_pb(3)
        if stage >= 4:
            job(0, 1)
            job(1, 0)
        proj_pb(4)
        proj_pb(5)
        if stage >= 4:
            job(0, 2)
        proj_pb(6)
        proj_pb(7)
        if stage >= 4:
            job(0, 3)
        b2_copies()
        wo_pre = []
        if stage >= 6:
            wo_sb_0 = pin.tile([128, 4 * E], F32R, tag="xin")
            wo_sb_1 = pin.tile([128, 4 * E], F32R, tag="xin")
            wo_pre = [wo_sb_0, wo_sb_1]
            for g in range(2):
                nc.sync.dma_start(wo_pre[g][:], wo[g])
        if stage >= 4:
            job(2, 0)
            job(1, 1)

        if dbg:
            nc.sync.dma_start(dbg_qt[:], QT[:].bitcast(F32))
            nc.sync.dma_start(dbg_kt[:], KT[:].bitcast(F32))
            nc.sync.dma_start(dbg_vt[:], VT[:].bitcast(F32))
            nc.sync.dma_start(dbg_q2[:], QT2[:].bitcast(F32))
            if stage >= 3:
                nc.sync.dma_start(dbg_acc[:], acc[:])
                nc.sync.dma_start(dbg_zz[:], zz[:])
        if stage <= 4:
            return

        # ---- normalization (reciprocal reshaped to use all 128 lanes) ----
        zw = persist.tile([128, 64], F32R, tag="zw")
        for i, zr in enumerate((0, 64)):
            nc.sync.dma_start(
                zw[:, 32 * i:32 * i + 32].bitcast(F32), zz[zr:zr + 1, :]
            )
        with nc.allow_low_precision(reason="tf32 norm"):
            nc.vector.reciprocal(zw[:], zw[:])
        for pb in range(NPB):
            rzp = opool.tile([2, PB], F32R, tag="rzp")
            nc.sync.dma_start(rzp[0:1, :], zw[16 * pb:16 * pb + 16, 0:32])
            nc.sync.dma_start(rzp[1:2, :], zw[16 * pb:16 * pb + 16, 32:64])
            rb = psw.tile([128, 1024], F32, tag="w")
            nc.tensor.matmul(
                rb[:, 0:PB], ind_sb[:], rzp[:], start=True, stop=True,
            )
            aslice = acc[:, pb * PB:(pb + 1) * PB]
            nc.vector.tensor_mul(aslice, aslice, rb[:, 0:PB])
            if stage >= 6 and NCH == 1:
                nc.sync.dma_start(
                    a2a_in[0][pb], acc[:, PB * pb:PB * (pb + 1)].bitcast(F32R)
                )
        if dbg and stage == 5:
            nc.sync.dma_start(dbg_acc[:], acc[:])
        if stage <= 5:
            return

        _staged = (NCH == 1)
        # ---- chunked AllToAll + output projection pipeline ----
        # split the position block into NCH chunks; a2a chunk h then overlaps
        # the output projection of chunk h-1.
        wo_sb = wo_pre
        CW = PB // NCH  # chunk width within each 512 block
        mg = persist.tile([128, 8 * PB], F32R, tag="acc")
        for h in range(NCH):
            if not _staged:
                for r in range(8):
                    nc.sync.dma_start(
                        a2a_in[h][r],
                        acc[:, PB * r + CW * h:PB * r + CW * (h + 1)].bitcast(F32R),
                    )
            nc.gpsimd.collective_compute(
                "AllToAll", mybir.AluOpType.bypass,
                replica_groups=[list(range(8))],
                ins=[a2a_in[h][:]], outs=[a2a_out[h][:]],
            )
            for s in range(8):
                nc.sync.dma_start(
                    mg[:, s * PB + CW * h:s * PB + CW * (h + 1)], a2a_out[h][s]
                )
            for ob in range(8):
                pt = psw.tile([128, 1024], F32, tag="w")
                for ec in range(KC):
                    w_t = wo_sb[ec // 4]
                    lhs = w_t[:, (ec % 4) * E + ob * 128:
                              (ec % 4) * E + (ob + 1) * 128]
                    nc.tensor.matmul(
                        pt[:, 0:CW], lhs,
                        mg[:, ec * PB + CW * h:ec * PB + CW * (h + 1)],
                        start=(ec == 0), stop=(ec == KC - 1),
                    )
                osb = opool.tile([128, CW], F32, tag="osb")
                nc.vector.tensor_scalar_add(
                    osb[:], pt[:, 0:CW], bo_sb[:, ob:ob + 1]
                )
                nc.sync.dma_start(
                    outT[ob * 128:(ob + 1) * 128, CW * h:CW * (h + 1)], osb[:]
                )

    with tile.TileContext(nc) as tc, contextlib.ExitStack() as ctx:
        _emit(tc, ctx)

    nc.compile()
    return nc


_NC_CACHE = {}


def _get_nc(stage=6, dbg=False):
    key = (stage, dbg)
    if key not in _NC_CACHE:
        _NC_CACHE[key] = _build(stage, dbg)
    return _NC_CACHE[key]


def _prep_inputs(query, key, value, Wq, bq, Wk, bk, Wv, bv, Wo, bo):
    """Host-side sharding/layout prep. Returns in_maps for the 8 cores."""
    qT = np.ascontiguousarray(query[0].T)  # (E, L)
    kT = np.ascontiguousarray(key[0].T)
    vT = np.ascontiguousarray(value[0].T)

    WqT = np.ascontiguousarray(Wq.T) * np.float32(0.125)
    WkT = np.ascontiguousarray(Wk.T)
    WvT = np.ascontiguousarray(Wv.T)

    # permuted Wo.T rows to match a2a feature order
    perm = np.concatenate(
        [np.r_[64 * s:64 * s + 64, 512 + 64 * s:512 + 64 * s + 64]
         for s in range(8)]
    )
    WoT = np.ascontiguousarray(Wo.T)[perm]  # (E e', E o)
    wo_pack = np.zeros((2, 128, 4 * E), np.float32)
    for ec in range(8):
        wo_pack[ec // 4, :, (ec % 4) * E:(ec % 4 + 1) * E] = \
            WoT[ec * 128:(ec + 1) * 128]

    bo_eff = (bo + bv @ Wo.T).astype(np.float32)
    bo8 = bo_eff.reshape(8, 128).T.copy()  # [p, ob]

    # per-core offset indicators: slot A offset = c//4 in {0,1} on rows 0-63,
    # slot B offset = 2 + c//4 (encoded as its low bit) on rows 64-127.
    WS = np.zeros((8, 128, 2), np.float32)
    for c in range(8):
        d = c // 4
        WS[c, 0:64, d] = 1.0
        WS[c, 64:128, d] = 1.0

    IND = np.zeros((2, 128), np.float32)
    IND[0, 0:64] = 1.0
    IND[1, 64:128] = 1.0
    EYE = np.eye(128, dtype=np.float32)
    ONES16 = np.ones((128, 16), np.float32)

    in_maps = []
    for c in range(8):
        fa = np.r_[64 * c:64 * c + 64]
        fb = np.r_[512 + 64 * c:512 + 64 * c + 64]
        sel = np.concatenate([fa, fb])
        in_maps.append({
            "qT": qT, "kT": kT, "vT": vT,
            "wq": np.ascontiguousarray(WqT[:, sel]),
            "wk": np.ascontiguousarray(WkT[:, sel]),
            "wv": np.ascontiguousarray(WvT[:, sel]),
            "wo": wo_pack,
            "bq": (bq[sel] * np.float32(0.125)).reshape(128, 1).astype(np.float32),
            "bk": bk[sel].reshape(128, 1).astype(np.float32),
            "bo8": bo8,
            "ind2": IND, "eyer": EYE, "ones16": ONES16,
            "wsel": WS[c],
        })
    return in_maps


def kernel(query, key, value, Wq, bq, Wk, bk, Wv, bv, Wo, bo,
           _trace=False, _result_holder=None, _stage=6, _dbg=False):
    args = [np.asarray(a, np.float32) for a in
            (query, key, value, Wq, bq, Wk, bk, Wv, bv, Wo, bo)]
    nc = _get_nc(_stage, _dbg)
    in_maps = _prep_inputs(*args)
    res = bass_utils.run_bass_kernel_spmd(
        nc, in_maps, core_ids=list(range(N_CORES)), trace=_trace
    )
    if _result_holder is not None:
        _result_holder.append(res)
    outT = np.zeros((E, L), np.float32)
    for c in range(N_CORES):
        outT[:, PB * c:PB * (c + 1)] = res.results[c]["outT"]
    return np.ascontiguousarray(outT.T).reshape(1, L, E)



# revision 12
# speedup vs baseline: 1.4367x; 1.4367x over previous
"""Dilated (LongNet-style) attention kernel for 8 TRN2 NeuronCores.

Head-sharded SPMD design (core c owns heads {c, 8+c}), bf16 datapath:
  - Inputs stream in as bf16 (halves HBM traffic); all matmuls run bf16
    (enables fast-weight-load, which fp32r denies) with fp32 PSUM.
  - Per (branch, segment) job: scores are computed transposed ([key, query])
    in 512-col PSUM banks; the causal mask of the diagonal 128-block is
    pre-loaded into PSUM by a tiny identity x TRI matmul that also clears the
    bank (start=True), so no post-exp masking op is needed. exp runs on ACT
    into bf16 SBUF tiles; AV accumulates (with an appended ones column in V
    supplying the softmax denominators Z) into fp32 PSUM.
  - Branch merge = raw sum of exp-weighted AV and Z across branches
    (softmax-of-lse merge is algebraically A_tot/Z_tot), scattered into a
    dense fp32 accumulator; single normalization at the end.
  - Projection matmuls are emitted as filler quanta *between* the attention
    jc-steps so the PE never idles while ACT computes exp.
  - Dilation-compressed copies for branch 2 run on GpSimd (core-dependent
    offsets handled by 0/1 indicator scalars, keeping the SPMD instruction
    stream core-uniform).
  - Tail: normalize -> bf16 AllToAll (feature->position redistribution) ->
    output projection per 512-position block.
"""

import sys

if "/opt/trn_rl_repo" not in sys.path:
    sys.path.insert(0, "/opt/trn_rl_repo")

import contextlib

import numpy as np
import ml_dtypes

import concourse.bacc as bacc
import concourse.bass as bass  # noqa: F401
import concourse.mybir as mybir
import concourse.tile as tile
from concourse import bass_utils

F32 = mybir.dt.float32
BF16 = mybir.dt.bfloat16
AF = mybir.ActivationFunctionType
BF = ml_dtypes.bfloat16

N_CORES = 8
E, L, H, D = 1024, 4096, 16, 64
KC = 8          # contraction chunks of 128 for the projections
PBP = 1024      # projection position block
NPP = L // PBP  # 4
PB = 512        # outproj position block (per core)
G = 1024        # compressed segment length (all branches)
VBW = 65        # V_both per-chunk width (64 feats + ones col)

# job order: last job only touches positions 2048-4095; br1 s0 after br0 s0/s1
JOBS = [(0, 0), (0, 1), (1, 0), (0, 2), (0, 3), (2, 0), (1, 1)]


def _build():
    nc = bacc.Bacc("TRN2", target_bir_lowering=False, debug=False,
                   num_devices=N_CORES)

    qT = nc.dram_tensor("qT", [E, L], BF16, kind="ExternalInput")
    kT = nc.dram_tensor("kT", [E, L], BF16, kind="ExternalInput")
    vT = nc.dram_tensor("vT", [E, L], BF16, kind="ExternalInput")
    w3 = nc.dram_tensor("w3", [128, 3 * KC * 128], BF16, kind="ExternalInput")
    wo = nc.dram_tensor("wo", [128, 8 * E], BF16, kind="ExternalInput")
    bqk = nc.dram_tensor("bqk", [128, 2], F32, kind="ExternalInput")
    bo8 = nc.dram_tensor("bo8", [128, 8], F32, kind="ExternalInput")
    trid = nc.dram_tensor("trid", [128, 128], BF16, kind="ExternalInput")
    eyed = nc.dram_tensor("eyed", [128, 128], BF16, kind="ExternalInput")
    onesd = nc.dram_tensor("onesd", [128, 16], BF16, kind="ExternalInput")
    wseld = nc.dram_tensor("wseld", [128, 2], F32, kind="ExternalInput")
    ind2d = nc.dram_tensor("ind2d", [2, 128], BF16, kind="ExternalInput")

    outT = nc.dram_tensor("outT", [E, PB], F32, kind="ExternalOutput")

    a2a_warm_in = nc.dram_tensor("a2a_warm_in", [8, 1, 64], BF16)
    a2a_warm_out = nc.dram_tensor("a2a_warm_out", [8, 1, 64], BF16)
    a2a_in = nc.dram_tensor("a2a_in", [8, 128, PB], BF16)
    a2a_out = nc.dram_tensor("a2a_out", [8, 128, PB], BF16)

    def _emit(tc, ctx):
        pin = ctx.enter_context(tc.tile_pool(name="pin", bufs=8))
        persist = ctx.enter_context(tc.tile_pool(name="persist", bufs=1))
        vpool = ctx.enter_context(tc.tile_pool(name="vpool", bufs=2))
        epool = ctx.enter_context(tc.tile_pool(name="epool", bufs=3))
        opool = ctx.enter_context(tc.tile_pool(name="opool", bufs=2))
        psS = ctx.enter_context(tc.tile_pool(name="psS", bufs=2, space="PSUM"))
        psO = ctx.enter_context(tc.tile_pool(name="psO", bufs=2, space="PSUM"))

        # ---- persistent tiles ----
        w3_sb = persist.tile([128, 3 * KC * 128], BF16, tag="w3")
        wo_sb = persist.tile([128, 8 * E], BF16, tag="wo")
        bqk_sb = persist.tile([128, 2], F32, tag="bqk")
        bo_sb = persist.tile([128, 8], F32, tag="bo")
        tri_sb = persist.tile([128, 128], BF16, tag="tri")
        eye_sb = persist.tile([128, 128], BF16, tag="eye")
        onz_sb = persist.tile([128, 16], BF16, tag="onz")
        ws_sb = persist.tile([128, 2], F32, tag="ws")
        ind_sb = persist.tile([2, 128], BF16, tag="ind")

        QT = persist.tile([128, L], BF16, tag="QT")
        KT = persist.tile([128, L], BF16, tag="KT")
        VT = persist.tile([128, L], BF16, tag="VT")
        QT2 = persist.tile([128, G], BF16, tag="QT2")
        KT2 = persist.tile([128, G], BF16, tag="KT2")
        VT2 = persist.tile([128, G], BF16, tag="VT2")
        acc = persist.tile([128, L], F32, tag="acc")
        zz = persist.tile([33, L], F32, tag="zz")
        accb = persist.tile([128, L], BF16, tag="accb")
        mg = persist.tile([128, 8 * PB], BF16, tag="mg")
        zw = persist.tile([128, 64], F32, tag="zw")
        zwb = persist.tile([128, 64], BF16, tag="zwb")

        # ---- constant loads (small, sync queue; wo on scalar) ----
        nc.scalar.dma_start(wo_sb[:], wo[:])
        nc.sync.dma_start(w3_sb[:], w3[:])
        nc.sync.dma_start(bqk_sb[:], bqk[:])
        nc.sync.dma_start(bo_sb[:], bo8[:])
        nc.sync.dma_start(tri_sb[:], trid[:])
        nc.sync.dma_start(eye_sb[:], eyed[:])
        nc.sync.dma_start(onz_sb[:], onesd[:])
        nc.sync.dma_start(ws_sb[:], wseld[:])
        nc.sync.dma_start(ind_sb[:], ind2d[:])

        # warm the ACT exp table early
        wtile = opool.tile([1, 16], BF16, tag="warm")
        nc.scalar.activation(wtile[:], onz_sb[0:1, 0:16], AF.Exp)

        # ---- stream all input position blocks up-front ----
        streams = (("k", kT, KT, 0), ("v", vT, VT, 1), ("q", qT, QT, 2))
        xin_tiles = {}
        for pb in range(NPP):
            for name, x_d, _, _ in streams:
                xr = x_d.rearrange("(kc p) l -> kc p l", p=128)
                for half in (0, 1):
                    xin = pin.tile([128, KC * 512], BF16, tag="xin")
                    c0 = pb * PBP + half * 512
                    for kc in range(KC):
                        nc.sync.dma_start(
                            xin[:, kc * 512:(kc + 1) * 512],
                            xr[kc][:, c0:c0 + 512],
                        )
                    xin_tiles[(pb, name, half)] = xin

        # ---- projection work quanta ----
        def proj_quantum(pb, sname, half):
            xin = xin_tiles[(pb, sname, half)]
            _, _, dst, soff = next(s for s in streams if s[0] == sname)
            pt = psS.tile([128, 512], F32, tag="ps")
            c0 = half * 512
            for kc in range(KC):
                nc.tensor.matmul(
                    pt[:, 0:512],
                    w3_sb[:, soff * 1024 + kc * 128:soff * 1024 + (kc + 1) * 128],
                    xin[:, kc * 512:(kc + 1) * 512],
                    start=(kc == 0), stop=(kc == KC - 1),
                )
            dslice = dst[:, pb * PBP + c0:pb * PBP + c0 + 512]
            if sname == "q":
                nc.vector.tensor_scalar_add(dslice, pt[:, 0:512], bqk_sb[:, 0:1])
            elif sname == "k":
                nc.vector.tensor_scalar_add(dslice, pt[:, 0:512], bqk_sb[:, 1:2])
            else:
                nc.vector.tensor_copy(dslice, pt[:, 0:512])

        def quanta_for_pb(pb):
            return [(pb, s, h) for s in ("k", "v", "q") for h in (0, 1)]

        # ---- branch-2 dilation-compressed copies (gpsimd, per pb chunk) ----
        def b2_part(pb):
            for src, dst in ((QT, QT2), (KT, KT2), (VT, VT2)):
                for slot in range(2):
                    p0 = 64 * slot
                    o0 = 2 * slot
                    dc = dst[p0:p0 + 64, pb * 256:(pb + 1) * 256]
                    s0 = pb * PBP + o0
                    nc.vector.tensor_scalar_mul(
                        dc,
                        src[p0:p0 + 64, s0:s0 + 4 * 255 + 1:4],
                        ws_sb[p0:p0 + 64, 0:1],
                    )
                    nc.vector.scalar_tensor_tensor(
                        dc,
                        src[p0:p0 + 64, s0 + 1:s0 + 1 + 4 * 255 + 1:4],
                        ws_sb[p0:p0 + 64, 1:2], dc,
                        mybir.AluOpType.mult, mybir.AluOpType.add,
                    )

        # ---- K/Q slicing per branch ----
        def kq_slice(br, seg, slot, t, lo, size):
            if br == 0:
                base = 1024 * seg + lo
                return t[slot * 64:(slot + 1) * 64, base:base + size]
            if br == 1:
                base = 2048 * seg + 2 * lo + slot
                return t[slot * 64:(slot + 1) * 64,
                         base:base + 2 * size - slot:2]
            return t[slot * 64:(slot + 1) * 64, lo:lo + size]

        # ---- one (branch, segment) job ----
        def job(br, seg, fillers):
            kt_src = KT2 if br == 2 else KT
            qt_src = QT2 if br == 2 else QT
            fillers = list(fillers)

            # V_both prep: PE transposes + DVE copies into vb
            vb = vpool.tile([128, 2 * 8 * VBW], BF16, tag="vb")
            nc.vector.tensor_copy(vb[:, 64::VBW], onz_sb[:])
            for jc in range(8):
                if br == 0:
                    tp = psS.tile([128, 128], BF16, tag="ps")
                    src = VT[:, 1024 * seg + 128 * jc:1024 * seg + 128 * (jc + 1)]
                    nc.tensor.transpose(tp[:, 0:128], src, eye_sb[:])
                    dst = vb[:].rearrange(
                        "p (s jj t) -> p s jj t", s=2, jj=8
                    )[:, :, jc, 0:64]
                    srcp = tp[:, 0:128].rearrange("p (s r) -> p s r", s=2)
                    nc.vector.tensor_copy(dst, srcp)
                else:
                    for slot in range(2):
                        tp = psS.tile([128, 128], BF16, tag="ps")
                        if br == 1:
                            base = 2048 * seg + 256 * jc + slot
                            src = VT[slot * 64:(slot + 1) * 64,
                                     base:base + 256 - slot:2]
                        else:
                            src = VT2[slot * 64:(slot + 1) * 64,
                                      128 * jc:128 * (jc + 1)]
                        nc.tensor.transpose(
                            tp[:, 0:64], src,
                            eye_sb[slot * 64:(slot + 1) * 64,
                                   slot * 64:(slot + 1) * 64],
                        )
                        nc.vector.tensor_copy(
                            vb[:, slot * 8 * VBW + jc * VBW:
                               slot * 8 * VBW + jc * VBW + 64],
                            tp[:, 0:64],
                        )

            o_ps_a = psO.tile([128, 1024], F32, tag="o")
            o_ps_b = psO.tile([128, 1024], F32, tag="o")
            o_ps = [o_ps_a, o_ps_b]

            def merge(r0, r1):
                # scatter o_ps[slot] region [r0:r1] (+ Z row) into acc/zz
                for slot in range(2):
                    op = o_ps[slot]
                    po = slot * 64
                    w = r1 - r0
                    if br == 0:
                        d0 = 1024 * seg + r0
                        nc.vector.tensor_copy(
                            acc[po:po + 64, d0:d0 + w], op[0:64, r0:r1])
                        zr = 32 * slot
                        nc.vector.tensor_copy(
                            zz[zr:zr + 1, d0:d0 + w], op[64:65, r0:r1])
                    elif br == 1:
                        d0 = 2048 * seg + 2 * r0 + slot
                        d1 = d0 + 2 * (w - 1) + 1
                        ac = acc[po:po + 64, d0:d1:2]
                        nc.vector.tensor_add(ac, ac, op[0:64, r0:r1])
                        zc = zz[32 * slot:32 * slot + 1, d0:d1:2]
                        nc.vector.tensor_add(zc, zc, op[64:65, r0:r1])
                    else:
                        o0 = 2 * slot
                        for dd in range(2):
                            d0 = 4 * r0 + o0 + dd
                            d1 = d0 + 4 * (w - 1) + 1
                            ac = acc[po:po + 64, d0:d1:4]
                            nc.vector.scalar_tensor_tensor(
                                ac, op[0:64, r0:r1], ws_sb[po:po + 64, dd:dd + 1],
                                ac, mybir.AluOpType.mult, mybir.AluOpType.add,
                            )
                            zr = 32 * slot
                            zc = zz[zr:zr + 1, d0:d1:4]
                            nc.vector.scalar_tensor_tensor(
                                zc, op[64:65, r0:r1], ws_sb[zr:zr + 1, dd:dd + 1],
                                zc, mybir.AluOpType.mult, mybir.AluOpType.add,
                            )

            for jc in range(8):
                c0 = 128 * jc
                es = []
                for slot in range(2):
                    s = psS.tile([128, 1024], F32, tag="ps")
                    lhs = kq_slice(br, seg, slot, kt_src, c0, 128)
                    # causal-mask bias first: clears the bank (start=True),
                    # writes -100 upper-triangle into the diagonal block.
                    nc.tensor.matmul(
                        s[:, c0:c0 + 128], eye_sb[:], tri_sb[:],
                        start=True, stop=False,
                    )
                    if c0 < 512:
                        nc.tensor.matmul(
                            s[:, c0:512], lhs,
                            kq_slice(br, seg, slot, qt_src, c0, 512 - c0),
                            start=False, stop=True,
                            tile_position=(slot * 64, 0),
                        )
                        nc.tensor.matmul(
                            s[:, 512:1024], lhs,
                            kq_slice(br, seg, slot, qt_src, 512, 512),
                            start=True, stop=True,
                            tile_position=(slot * 64, 0),
                        )
                    else:
                        nc.tensor.matmul(
                            s[:, c0:1024], lhs,
                            kq_slice(br, seg, slot, qt_src, c0, 1024 - c0),
                            start=False, stop=True,
                            tile_position=(slot * 64, 0),
                        )
                    e = epool.tile([128, 1024], BF16, tag="e")
                    nc.scalar.activation(e[:, c0:1024], s[:, c0:1024], AF.Exp)
                    es.append(e)

                if fillers:
                    pbq, sq, hq = fillers.pop(0)
                    proj_quantum(pbq, sq, hq)

                for slot in range(2):
                    e = es[slot]
                    vbs = vb[:, slot * 8 * VBW + jc * VBW:
                             slot * 8 * VBW + (jc + 1) * VBW]
                    if c0 < 512:
                        nc.tensor.matmul(
                            o_ps[slot][0:VBW, c0:512], vbs, e[:, c0:512],
                            start=(jc == 0), stop=(jc == 3),
                        )
                        nc.tensor.matmul(
                            o_ps[slot][0:VBW, 512:1024], vbs, e[:, 512:1024],
                            start=(jc == 0), stop=(jc == 7),
                        )
                    else:
                        nc.tensor.matmul(
                            o_ps[slot][0:VBW, c0:1024], vbs, e[:, c0:1024],
                            start=(jc == 0), stop=(jc == 7),
                        )
                if jc == 3:
                    merge(0, 512)
            merge(512, 1024)
            # drain any leftover fillers
            for pbq, sq, hq in fillers:
                proj_quantum(pbq, sq, hq)

        # ================= emission order =================
        for q in quanta_for_pb(0):
            proj_quantum(*q)
        # tiny dummy collective absorbs first-collective setup cost
        # (staging on the scalar queue so it isn't stuck behind xin triggers)
        for rr in range(8):
            nc.scalar.dma_start(a2a_warm_in[rr][0:1, 0:16], onz_sb[0:1, 0:16])
        nc.gpsimd.collective_compute(
            "AllToAll", mybir.AluOpType.bypass,
            replica_groups=[list(range(8))],
            ins=[a2a_warm_in[:]], outs=[a2a_warm_out[:]],
        )
        b2_part(0)
        job(0, 0, quanta_for_pb(1))
        b2_part(1)
        job(0, 1, quanta_for_pb(2))
        b2_part(2)
        job(1, 0, quanta_for_pb(3))
        b2_part(3)
        job(0, 2, [])
        job(0, 3, [])
        job(2, 0, [])
        job(1, 1, [])

        # ---- normalization ----
        for i in range(2):
            nc.sync.dma_start(zw[:, 32 * i:32 * i + 32], zz[32 * i:32 * i + 1, :])
        with nc.allow_low_precision(reason="softmax denom reciprocal"):
            nc.vector.reciprocal(zw[:], zw[:])
        nc.vector.tensor_copy(zwb[:], zw[:])
        for pb in range(8):
            rzp = opool.tile([2, PB], BF16, tag="rzp")
            nc.sync.dma_start(rzp[0:1, :], zwb[16 * pb:16 * pb + 16, 0:32])
            nc.sync.dma_start(rzp[1:2, :], zwb[16 * pb:16 * pb + 16, 32:64])
            rb = psS.tile([128, 512], F32, tag="ps")
            nc.tensor.matmul(rb[:, 0:512], ind_sb[:], rzp[:],
                             start=True, stop=True)
            with nc.allow_low_precision(reason="bf16 a2a payload"):
                nc.vector.tensor_mul(
                    accb[:, pb * PB:(pb + 1) * PB],
                    acc[:, pb * PB:(pb + 1) * PB], rb[:, 0:512])
            nc.sync.dma_start(a2a_in[pb], accb[:, pb * PB:(pb + 1) * PB])

        # ---- AllToAll + output projection ----
        nc.gpsimd.collective_compute(
            "AllToAll", mybir.AluOpType.bypass,
            replica_groups=[list(range(8))],
            ins=[a2a_in[:]], outs=[a2a_out[:]],
        )
        for s in range(8):
            nc.sync.dma_start(mg[:, s * PB:(s + 1) * PB], a2a_out[s])
        for ob in range(8):
            pt = psS.tile([128, 512], F32, tag="ps")
            for ec in range(KC):
                nc.tensor.matmul(
                    pt[:, 0:512],
                    wo_sb[:, ec * E + ob * 128:ec * E + (ob + 1) * 128],
                    mg[:, ec * PB:(ec + 1) * PB],
                    start=(ec == 0), stop=(ec == KC - 1),
                )
            osb = opool.tile([128, PB], F32, tag="osb")
            nc.vector.tensor_scalar_add(osb[:], pt[:, 0:512], bo_sb[:, ob:ob + 1])
            nc.sync.dma_start(outT[ob * 128:(ob + 1) * 128, :], osb[:])

    with tile.TileContext(nc) as tc, contextlib.ExitStack() as ctx:
        _emit(tc, ctx)

    nc.compile()
    return nc


_NC_CACHE = {}


def _get_nc():
    if "nc" not in _NC_CACHE:
        _NC_CACHE["nc"] = _build()
    return _NC_CACHE["nc"]


def _prep_inputs(query, key, value, Wq, bq, Wk, bk, Wv, bv, Wo, bo):
    """Host-side layout prep. Returns in_maps for the 8 cores."""
    qT = np.ascontiguousarray(query[0].T).astype(BF)   # (E, L)
    kT = np.ascontiguousarray(key[0].T).astype(BF)
    vT = np.ascontiguousarray(value[0].T).astype(BF)

    WqT = np.ascontiguousarray(Wq.T) * np.float32(0.125)
    WkT = np.ascontiguousarray(Wk.T)
    WvT = np.ascontiguousarray(Wv.T)

    # permuted Wo.T rows to match a2a arriving-feature order
    perm = np.concatenate(
        [np.r_[64 * s:64 * s + 64, 512 + 64 * s:512 + 64 * s + 64]
         for s in range(8)]
    )
    WoT = np.ascontiguousarray(Wo.T)[perm]            # (E e', E o)
    wo_pack = np.zeros((128, 8 * E), np.float32)
    for ec in range(8):
        wo_pack[:, ec * E:(ec + 1) * E] = WoT[ec * 128:(ec + 1) * 128]
    wo_pack = wo_pack.astype(BF)

    bo_eff = (bo + bv @ Wo.T).astype(np.float32)
    bo8 = bo_eff.reshape(8, 128).T.copy()             # [p, ob]

    # per-core offset indicators for branch 2 (slot A offset c//4,
    # slot B offset 2 + c//4; low bit selected via indicators)
    WS = np.zeros((8, 128, 2), np.float32)
    for c in range(8):
        d = c // 4
        WS[c, :, d] = 1.0

    IND = np.zeros((2, 128), np.float32)
    IND[0, 0:64] = 1.0
    IND[1, 64:128] = 1.0
    # [k, q] layout: mask where q < k
    TRI = np.zeros((128, 128), np.float32)
    ki, qi = np.meshgrid(np.arange(128), np.arange(128), indexing="ij")
    TRI[qi < ki] = -100.0
    EYE = np.eye(128, dtype=np.float32)
    ONESZ = np.ones((128, 16), np.float32)

    in_maps = []
    for c in range(8):
        fa = np.r_[64 * c:64 * c + 64]
        fb = np.r_[512 + 64 * c:512 + 64 * c + 64]
        sel = np.concatenate([fa, fb])
        w3 = np.zeros((128, 3 * KC * 128), np.float32)
        for soff, WT in ((0, WkT), (1, WvT), (2, WqT)):
            for kc in range(KC):
                w3[:, soff * 1024 + kc * 128:soff * 1024 + (kc + 1) * 128] = \
                    WT[kc * 128:(kc + 1) * 128][:, sel]
        bqk_c = np.stack(
            [bq[sel] * np.float32(0.125), bk[sel]], axis=1
        ).astype(np.float32)
        in_maps.append({
            "qT": qT, "kT": kT, "vT": vT,
            "w3": w3.astype(BF),
            "wo": wo_pack,
            "bqk": bqk_c,
            "bo8": bo8,
            "trid": TRI.astype(BF),
            "eyed": EYE.astype(BF),
            "onesd": ONESZ.astype(BF),
            "wseld": WS[c],
            "ind2d": IND.astype(BF),
        })
    return in_maps


def kernel(query, key, value, Wq, bq, Wk, bk, Wv, bv, Wo, bo,
           _trace=False, _result_holder=None):
    args = [np.asarray(a, np.float32) for a in
            (query, key, value, Wq, bq, Wk, bk, Wv, bv, Wo, bo)]
    nc = _get_nc()
    in_maps = _prep_inputs(*args)
    res = bass_utils.run_bass_kernel_spmd(
        nc, in_maps, core_ids=list(range(N_CORES)), trace=_trace
    )
    if _result_holder is not None:
        _result_holder.append(res)
    outT = np.zeros((E, L), np.float32)
    for c in range(N_CORES):
        outT[:, PB * c:PB * (c + 1)] = res.results[c]["outT"]
    return np.ascontiguousarray(outT.T).reshape(1, L, E)
